# revision 57
# baseline (speedup 1.0000x reference)
"""Trainium2 Bass kernel for nn_MultiHeadAttention_88536455840315 (v2).

Math notes (vs the jax reference):
  - The second einsum (log_probs[..., None] * attn) @ v factors to
    log_probs[..., None] * (attn @ v) because log_probs does not depend on
    the key index.  Only two big attention matmuls are needed.
  - Softmax without max subtraction: dots*scale ~ N(0,1), exp never
    overflows fp32/bf16.
  - sumexp is fused into attn@v as a ones column appended to V.

Sharding (8 cores): core c handles batch c//4 and query rows
(c%4)*512 .. +512 (host rolls the batch so queries are rows 0:512).
Each core computes full K/V for its batch (replicated in the 4-core
group, no collectives - the cost model prices collectives at 15us
constant overhead, far too slow here).

v2 changes vs the 238us baseline:
  - All tensor data in bf16 (host casts x and the weights); fp32 only in
    PSUM accumulators and the statistics tail.  Host-side casting halves
    DMA traffic and removes all on-chip weight conversions.
  - attn@V runs in [q, 65] orientation (65 = DH + sumexp column): the
    ragged 65 sits on the free dim, so it costs 65 cycles per (kb, qc)
    instead of wasting half the PE partitions; it also writes PROD in
    its natural layout, killing the per-head [65,512] transposes.
  - softmax exp is split across ACT and DVE (GPSIMD cannot read PSUM):
    ACT runs native Exp, DVE runs a one-instruction Schraudolph exp
    (bf16 bits = round(x*scale*log2e*128 + (127-sigma)*128) computed as
    int16 and bitcast) - ~1% rms error, which washes out over the
    softmax sum.  Two dots chunks share one PSUM tile and one 1024-elem
    exp instruction to halve per-op overheads.
  - the emission schedule software-pipelines everything: kq slot c+1 and
    dots pair c+1 are emitted before attn@V of pair c, the V projection
    fills the first exp wait, the two heads of a pair interleave their
    attn@V chains, and the cross-head sum/sum-of-squares accumulate on
    the otherwise idle GPSIMD during attention so only the variance ->
    log-prob -> output-projection chain remains at the end (pipelined
    per query tile).
  - fp8/DoubleRow, DMA-XBAR transposes, collectives and remote-DMA K/V
    exchange were all evaluated and rejected: fp8 breaks the 2e-2 error
    budget (max-over-3M-outputs ~ 5 sigma of the quantization noise),
    the cost model prices collectives at 15us+ constant overhead, XBAR
    transfers with consumers serialize at ~8us each in the sim, and the
    Tile scheduler deadlocks on remote-semaphore waits.
"""

import sys

if "/opt/trn_rl_repo" not in sys.path:
    sys.path.insert(0, "/opt/trn_rl_repo")

import numpy as np
import ml_dtypes

import concourse.bass as bass
import concourse.mybir as mybir
import concourse.tile as tile
from concourse import bacc
from concourse import bass_utils
from concourse.masks import make_identity

F32 = mybir.dt.float32
BF16 = mybir.dt.bfloat16
I16 = mybir.dt.int16
AF = mybir.ActivationFunctionType
ALU = mybir.AluOpType
AX = mybir.AxisListType

B, N, E = 2, 2048, 768
H, DH = 12, 64
HD = H * DH            # 768
NQ = 512               # query rows per core
SCALE = DH ** -0.5
LOG2PI = float(np.log(2.0 * np.pi))
CONST = -0.5 * DH * LOG2PI   # -32*log(2*pi)
LOG2E = float(np.log2(np.e))

NE = E // 128          # 6 chunks of the embedding dim
NN = N // 128          # 16 chunks of the sequence
NQT = NQ // 128        # 4 query tiles

# Schraudolph exp -> bf16 bit pattern, computed as int16 on DVE.
# (GPSIMD cannot read PSUM - neuronxcc rejects it - so exp is ACT+DVE only.)
# The -1.43 trims the +0.78% bias measured on hardware.
EXP_A = SCALE * LOG2E * 128.0
EXP_B = (127.0 - 0.0436) * 128.0 - 1.43


def _emit(tc):
    nc = tc.nc
    xb = nc.dram_tensor("xb", [N, E], BF16, kind="ExternalInput").ap()
    wq = nc.dram_tensor("wq", [E, E], BF16, kind="ExternalInput").ap()
    wk = nc.dram_tensor("wk", [E, E], BF16, kind="ExternalInput").ap()
    wv = nc.dram_tensor("wv", [E, E], BF16, kind="ExternalInput").ap()
    wo = nc.dram_tensor("wo", [E, E], BF16, kind="ExternalInput").ap()
    bout_t = nc.dram_tensor("bout", [E], F32, kind="ExternalInput")
    y = nc.dram_tensor("y", [NQ, E], F32, kind="ExternalOutput").ap()

    # exp engine schedule: only ACT and DVE can read PSUM; balanced 1:1
    # because DVE also carries the QT drains and normalization.
    exp_sched = ["A", "D"]

    with tc.tile_pool(name="consts", bufs=1) as consts, \
         tc.tile_pool(name="big", bufs=1) as big:
        identf = consts.tile([128, 128], F32, name="identf", tag="identf")
        make_identity(nc, identf)
        ident = consts.tile([128, 128], BF16, name="ident", tag="ident")
        nc.vector.tensor_copy(ident, identf)
        bias = consts.tile([128, E], F32, name="bias", tag="bias")
        nc.scalar.dma_start(out=bias, in_=bass.AP(
            tensor=bout_t, offset=0, ap=[[0, 128], [1, E]]))

        XT = [[big.tile([128, 512], BF16, name=f"xt{e}_{g}",
                        tag=f"xt{e}_{g}") for g in range(4)]
              for e in range(NE)]
        KT = [big.tile([128, N], BF16, name=f"kt{c}", tag=f"kt{c}")
              for c in range(NE)]
        QT = [big.tile([128, NQ], BF16, name=f"qt{c}", tag=f"qt{c}")
              for c in range(NE)]
        VA = [big.tile([128, H, DH + 1], BF16, name=f"va{j}", tag=f"va{j}")
              for j in range(NN)]
        PROD = big.tile([128, NQT, H, DH], F32, name="prod", tag="prod")
        ACCS = big.tile([128, NQT, DH], F32, name="accs", tag="accs")
        ACCQ = big.tile([128, NQT, DH], F32, name="accq", tag="accq")
        SQT = big.tile([128, NQT, DH], F32, name="sqt", tag="sqt")
        WOS = [big.tile([128, E], BF16, name=f"wos{e}", tag=f"wos{e}")
               for e in range(NE)]

        with tc.tile_pool(name="wsb", bufs=1) as wsb:
            WKS = [wsb.tile([128, E], BF16, name=f"wks{e}", tag=f"wks{e}")
                   for e in range(NE)]
            WQS = [wsb.tile([128, E], BF16, name=f"wqs{e}", tag=f"wqs{e}")
                   for e in range(NE)]
            WVS = [wsb.tile([128, E], BF16, name=f"wvs{e}", tag=f"wvs{e}")
                   for e in range(NE)]

            # front DMAs.  x rides the sync queue first (it gates the
            # transposes and every projection), wk/wq follow there; wv/wo/
            # bias ride the scalar queue.  (The DMA XBAR transpose was tried
            # for x^T but the sim serializes XBAR transfers that have
            # consumers at ~8us each - PE transposes are far cheaper.)
            xsp_ctx = tc.tile_pool(name="xsp", bufs=1)
            xsp = xsp_ctx.__enter__()
            XS = [xsp.tile([128, E], BF16, name=f"xs{j}", tag=f"xs{j}")
                  for j in range(NN)]
            for j in range(NN):
                nc.sync.dma_start(out=XS[j],
                                  in_=xb[j * 128:(j + 1) * 128, :])
            for e in range(NE):
                nc.sync.dma_start(out=WKS[e],
                                  in_=wk[e * 128:(e + 1) * 128, :])
            for e in range(NE):
                nc.sync.dma_start(out=WQS[e],
                                  in_=wq[e * 128:(e + 1) * 128, :])
            for e in range(NE):
                nc.scalar.dma_start(out=WVS[e],
                                    in_=wv[e * 128:(e + 1) * 128, :])
            for e in range(NE):
                nc.scalar.dma_start(out=WOS[e],
                                    in_=wo[e * 128:(e + 1) * 128, :])

            # ones columns of VA (sumexp trick)
            for j in range(NN):
                nc.gpsimd.memset(bass.AP(
                    tensor=VA[j].tensor, offset=VA[j].offset + DH,
                    ap=[VA[j].ap[0], [DH + 1, H], [1, 1]]), 1.0)

            # x^T via PE transposes, 4 n-blocks per PSUM bank; the XS
            # staging pool closes right after so its SBUF is reused by expp.
            with tc.tile_pool(name="tpp", bufs=2, space="PSUM") as tpp:
                for nbg in range(4):
                    for e in range(NE):
                        pt = tpp.tile([128, 512], BF16, name="tp", tag="tp")
                        for k in range(4):
                            nb = nbg * 4 + k
                            nc.tensor.transpose(
                                pt[:, k * 128:(k + 1) * 128],
                                XS[nb][:, e * 128:(e + 1) * 128], ident)
                        dst = XT[e][nbg]
                        if (nbg + e) % 2:
                            nc.scalar.copy(dst, pt)
                        else:
                            nc.vector.tensor_copy(dst, pt)
            xsp_ctx.__exit__(None, None, None)

            with tc.tile_pool(name="pp", bufs=2, space="PSUM") as pp, \
                 tc.tile_pool(name="dp", bufs=2, space="PSUM") as dp, \
                 tc.tile_pool(name="avp", bufs=2, space="PSUM") as avp, \
                 tc.tile_pool(name="expp", bufs=2) as expp, \
                 tc.tile_pool(name="nsb", bufs=4) as nsb:

                vnb = [0]   # next V-projection block to emit

                def v_proj_blocks(n_blocks):
                    for _ in range(n_blocks):
                        nb = vnb[0]
                        if nb >= NN:
                            return
                        vnb[0] += 1
                        psA = pp.tile([128, 512], F32, name="vpa", tag="pp")
                        va = VA[nb]
                        for e in range(NE):
                            nc.tensor.matmul(
                                psA,
                                XT[e][nb // 4][:, (nb % 4) * 128:
                                               (nb % 4 + 1) * 128],
                                WVS[e][:, 0:512],
                                start=(e == 0), stop=(e == NE - 1))
                        nc.scalar.copy(
                            bass.AP(tensor=va.tensor, offset=va.offset,
                                    ap=[va.ap[0], [DH + 1, 8], [1, DH]]),
                            psA.rearrange("p (h d) -> p h d", h=8))
                        psB = pp.tile([128, 256], F32, name="vpb", tag="pp")
                        for e in range(NE):
                            nc.tensor.matmul(
                                psB,
                                XT[e][nb // 4][:, (nb % 4) * 128:
                                               (nb % 4 + 1) * 128],
                                WVS[e][:, 512:768],
                                start=(e == 0), stop=(e == NE - 1))
                        nc.vector.tensor_copy(
                            bass.AP(tensor=va.tensor,
                                    offset=va.offset + 8 * (DH + 1),
                                    ap=[va.ap[0], [DH + 1, 4], [1, DH]]),
                            psB.rearrange("p (h d) -> p h d", h=4))

                EXPT = {}

                def dots_pair(h0, h1):
                    # two kb chunks share one PSUM tile and one exp
                    # instruction (1024 elems) to halve per-op overheads
                    for h in (h0, h1):
                        EXPT[h] = []
                    for kbp in range(NN // 2):
                        for i, h in enumerate((h0, h1)):
                            c, pofs = h // 2, (h % 2) * DH
                            d = dp.tile([128, 2, 512], F32, name="dots",
                                        tag="dots")
                            for t in range(2):
                                kb = 2 * kbp + t
                                nc.tensor.matmul(
                                    d[:, t, :],
                                    KT[c][pofs:pofs + DH,
                                          kb * 128:(kb + 1) * 128],
                                    QT[c][pofs:pofs + DH, :],
                                    start=True, stop=True)
                            ex = expp.tile([128, 2, 512], I16, name="ex",
                                           tag=f"ex{kbp}x{i}")
                            if (kbp + i) % 2 == 0:
                                nc.scalar.activation(out=ex.bitcast(BF16),
                                                     in_=d, func=AF.Exp,
                                                     scale=SCALE)
                            else:
                                nc.vector.tensor_scalar(
                                    out=ex, in0=d, scalar1=EXP_A,
                                    scalar2=EXP_B,
                                    op0=ALU.mult, op1=ALU.add)
                            EXPT[h].append(ex)

                def attnv_one(h, tiles, av, qc):
                    for kb in range(NN):
                        nc.tensor.matmul(
                            av[:, qc, :], tiles[kb // 2].bitcast(BF16)[
                                :, kb % 2,
                                qc * 128:(qc + 1) * 128],
                            VA[kb][:, h, :],
                            start=(kb == 0), stop=(kb == NN - 1))
                    # normalize: PROD[:, qc, h, :] = av[:, 0:64]/av[:, 64]
                    rq = nsb.tile([128, 1], F32, name="rq", tag="rq",
                                  bufs=8)
                    nc.vector.reciprocal(rq, av[:, qc, DH:DH + 1])
                    rq_bc = bass.AP(tensor=rq.tensor, offset=rq.offset,
                                    ap=[rq.ap[0], [0, DH]])
                    nc.vector.tensor_tensor(out=PROD[:, qc, h, :],
                                            in0=av[:, qc, 0:DH],
                                            in1=rq_bc,
                                            op=ALU.mult)

                def attnv_pair(h0, h1):
                    t0, t1 = EXPT.pop(h0), EXPT.pop(h1)
                    av0 = avp.tile([128, NQT, DH + 1], F32, name="av",
                                   tag="av")
                    av1 = avp.tile([128, NQT, DH + 1], F32, name="av2",
                                   tag="av")
                    for qc in range(NQT):
                        attnv_one(h0, t0, av0, qc)
                        attnv_one(h1, t1, av1, qc)
                    # running cross-head sums on the otherwise idle GPSIMD
                    for h in (h0, h1):
                        pvh = bass.AP(tensor=PROD.tensor,
                                      offset=PROD.offset + h * DH,
                                      ap=[PROD.ap[0], [H * DH, NQT],
                                          [1, DH]])
                        if h == 0:
                            nc.gpsimd.tensor_copy(ACCS, pvh)
                            nc.gpsimd.tensor_tensor(out=ACCQ, in0=pvh,
                                                    in1=pvh, op=ALU.mult)
                        else:
                            nc.gpsimd.tensor_tensor(out=ACCS, in0=ACCS,
                                                    in1=pvh, op=ALU.add)
                            nc.gpsimd.tensor_tensor(out=SQT, in0=pvh,
                                                    in1=pvh, op=ALU.mult)
                            nc.gpsimd.tensor_tensor(out=ACCQ, in0=ACCQ,
                                                    in1=SQT, op=ALU.add)

                def kq_slot(c):
                    for nb4 in range(4):
                        ps = pp.tile([128, 512], F32, name="kp", tag="pp")
                        for e in range(NE):
                            nc.tensor.matmul(
                                ps, WKS[e][:, c * 128:(c + 1) * 128],
                                XT[e][nb4],
                                start=(e == 0), stop=(e == NE - 1))
                        if nb4 % 2:
                            nc.vector.tensor_copy(
                                KT[c][:, nb4 * 512:(nb4 + 1) * 512], ps)
                        else:
                            nc.scalar.copy(
                                KT[c][:, nb4 * 512:(nb4 + 1) * 512], ps)
                    ps = pp.tile([128, 512], F32, name="qp", tag="pp")
                    for e in range(NE):
                        nc.tensor.matmul(
                            ps, WQS[e][:, c * 128:(c + 1) * 128],
                            XT[e][0],
                            start=(e == 0), stop=(e == NE - 1))
                    nc.scalar.copy(QT[c], ps)

                # ---- interleaved projection + attention schedule ----
                # kq_slot(c+1) is emitted one head-pair early so its KT/QT
                # drains clear the ACT/DVE queues before dots(2c+2) needs
                # them.
                # dots_pair(c+1) is emitted before attnv(pair c) so PE has
                # dots work queued while pair c's exp completes on ACT/DVE.
                kq_slot(0)
                kq_slot(1)
                dots_pair(0, 1)
                v_proj_blocks(NN)      # V projection fills the exp wait
                kq_slot(2)
                dots_pair(2, 3)
                attnv_pair(0, 1)
                for c in range(2, NE):
                    if c + 1 < NE:
                        kq_slot(c + 1)
                    dots_pair(2 * c, 2 * c + 1)
                    attnv_pair(2 * c - 4 + 2, 2 * c - 4 + 3)
                attnv_pair(10, 11)

        # ------- statistics / log-prob weighting / output projection -------
        # ACCS/ACCQ were accumulated per head on GPSIMD during attention, so
        # only the variance math and the per-qt chains remain; each qt chain
        # feeds its transposes + output projection immediately so the PE tail
        # overlaps the DVE/ACT statistics of the next qt.
        with tc.tile_pool(name="ohp", bufs=1) as ohp, \
             tc.tile_pool(name="wkp", bufs=1) as wkp, \
             tc.tile_pool(name="finp", bufs=2) as finp, \
             tc.tile_pool(name="tp2", bufs=2, space="PSUM") as tp2p, \
             tc.tile_pool(name="fps", bufs=3, space="PSUM") as fps:
            OH = ohp.tile([128, NQT, HD], BF16, name="oh", tag="oh")
            OHT = [ohp.tile([128, NE, 128], BF16, name=f"oht{qt}",
                            tag=f"oht{qt}") for qt in range(NQT)]
            mean = wkp.tile([128, NQT, DH], F32, name="mean", tag="mean")
            nc.vector.tensor_scalar_mul(mean, ACCS, 1.0 / H)
            m2s = wkp.tile([128, NQT, DH], F32, name="m2s", tag="m2s")
            nc.scalar.activation(out=m2s, in_=mean, func=AF.Square,
                                 scale=float(np.sqrt(H / (H - 1.0))))
            var = wkp.tile([128, NQT, DH], F32, name="var", tag="var")
            nc.vector.scalar_tensor_tensor(out=var, in0=ACCQ,
                                           scalar=1.0 / (H - 1), in1=m2s,
                                           op0=ALU.mult, op1=ALU.subtract)
            # qt chains run pair-interleaved: while ACT squares qt's diff,
            # DVE already runs qt+1's preamble/subtract.
            def chain_front(qt):
                pvq = bass.AP(tensor=PROD.tensor,
                              offset=PROD.offset + qt * H * DH,
                              ap=[PROD.ap[0], [DH, H], [1, DH]])
                varq = var[:, qt, :]
                rvar = wkp.tile([128, DH], F32, name="rvar", tag="rvar",
                                bufs=2)
                nc.vector.reciprocal(rvar, varq)
                lv = wkp.tile([128, DH], F32, name="lv", tag="lv", bufs=2)
                S = wkp.tile([128, 1], F32, name="S", tag="S", bufs=2)
                nc.scalar.activation(out=lv, in_=varq, func=AF.Ln,
                                     accum_out=S)
                cs = wkp.tile([128, 1], F32, name="cs", tag="cs", bufs=2)
                nc.vector.tensor_scalar(out=cs, in0=S, scalar1=-1.0,
                                        scalar2=CONST, op0=ALU.mult,
                                        op1=ALU.add)
                diff = wkp.tile([128, H, DH], F32, name="diff", tag="diff",
                                bufs=2)
                mean_bc = bass.AP(tensor=mean.tensor,
                                  offset=mean.offset + qt * DH,
                                  ap=[mean.ap[0], [0, H], [1, DH]])
                nc.vector.tensor_tensor(out=diff, in0=pvq, in1=mean_bc,
                                        op=ALU.subtract)
                nc.scalar.activation(out=diff, in_=diff, func=AF.Square)
                return pvq, rvar, cs, diff

            def chain_back(qt, st):
                pvq, rvar, cs, diff = st
                rvar_bc = bass.AP(tensor=rvar.tensor, offset=rvar.offset,
                                  ap=[rvar.ap[0], [0, H], [1, DH]])
                nc.vector.tensor_tensor(out=diff, in0=diff, in1=rvar_bc,
                                        op=ALU.mult)
                lp0 = wkp.tile([128, H], F32, name="lp0", tag="lp0", bufs=2)
                nc.vector.reduce_sum(lp0, diff, axis=AX.X)
                lp = wkp.tile([128, H], F32, name="lp", tag="lp", bufs=2)
                nc.vector.tensor_scalar(out=lp, in0=lp0, scalar1=0.25,
                                        scalar2=cs, op0=ALU.mult, op1=ALU.add)
                ohv = OH[:, qt, :].rearrange("p (h d) -> p h d", h=H)
                lp_bc = bass.AP(tensor=lp.tensor, offset=lp.offset,
                                ap=[lp.ap[0], [1, H], [0, DH]])
                nc.vector.tensor_tensor(out=ohv, in0=pvq, in1=lp_bc,
                                        op=ALU.mult)

            def out_proj(qt):
                for c in range(NE):
                    tp = tp2p.tile([128, 128], BF16, name="t2", tag="t2")
                    nc.tensor.transpose(
                        tp, OH[:, qt, c * 128:(c + 1) * 128], ident)
                    if (qt + c) % 2:
                        nc.scalar.copy(OHT[qt][:, c, :], tp)
                    else:
                        nc.vector.tensor_copy(OHT[qt][:, c, :], tp)
                psA = fps.tile([128, 512], F32, name="fA", tag="f")
                psB = fps.tile([128, 256], F32, name="fB", tag="f")
                for c in range(NE):
                    nc.tensor.matmul(psA, OHT[qt][:, c, :], WOS[c][:, 0:512],
                                     start=(c == 0), stop=(c == NE - 1))
                for c in range(NE):
                    nc.tensor.matmul(psB, OHT[qt][:, c, :],
                                     WOS[c][:, 512:768],
                                     start=(c == 0), stop=(c == NE - 1))
                fin = finp.tile([128, E], F32, name="fin", tag="fin")
                nc.vector.tensor_tensor(out=fin[:, 0:512], in0=psA,
                                        in1=bias[:, 0:512], op=ALU.add)
                nc.vector.tensor_tensor(out=fin[:, 512:768], in0=psB,
                                        in1=bias[:, 512:768], op=ALU.add)
                nc.sync.dma_start(out=y[qt * 128:(qt + 1) * 128, :],
                                  in_=fin)

            for qt in range(NQT):
                chain_back(qt, chain_front(qt))
                out_proj(qt)


_NC_CACHE = {}


def _get_nc():
    if "nc" not in _NC_CACHE:
        nc = bacc.Bacc("TRN2", target_bir_lowering=False, debug=False,
                       num_devices=8)
        with tile.TileContext(nc) as tc:
            _emit(tc)
        nc.compile()
        _NC_CACHE["nc"] = nc
    return _NC_CACHE["nc"]


def kernel(x, w_qkv, w_out, b_out):
    x = np.ascontiguousarray(x, dtype=np.float32)
    w_qkv = np.ascontiguousarray(w_qkv, dtype=np.float32)
    w_out = np.ascontiguousarray(w_out, dtype=np.float32)
    b_out = np.ascontiguousarray(b_out, dtype=np.float32)
    assert x.shape == (B, N, E)

    bf = ml_dtypes.bfloat16
    wqb = np.ascontiguousarray(w_qkv[:, 0:HD]).astype(bf)
    wkb = np.ascontiguousarray(w_qkv[:, HD:2 * HD]).astype(bf)
    wvb = np.ascontiguousarray(w_qkv[:, 2 * HD:3 * HD]).astype(bf)
    wob = w_out.astype(bf)

    nc = _get_nc()
    in_maps = []
    for c in range(8):
        beta, qoff = c // 4, (c % 4) * NQ
        xbc = np.ascontiguousarray(np.roll(x[beta], -qoff, axis=0)).astype(bf)
        in_maps.append({"xb": xbc, "wq": wqb, "wk": wkb, "wv": wvb,
                       "wo": wob, "bout": b_out})
    res = bass_utils.run_bass_kernel_spmd(nc, in_maps, core_ids=list(range(8)))
    out = np.empty((B, N, E), dtype=np.float32)
    for c in range(8):
        beta, qoff = c // 4, (c % 4) * NQ
        out[beta, qoff:qoff + NQ, :] = res.results[c]["y"]
    return out
